# revision 18
# baseline (speedup 1.0000x reference)
"""AngularLoss Trainium2 kernel (8 NeuronCores, SPMD data-parallel).

Computation (reference):
    t2  = tan(alpha_deg * pi/180)^2
    apn = rowsum((a + p) * n)          # [N,1]
    ap  = rowsum(a * p)                # [N,1]
    f   = 4*t2*apn - 2*(1+t2)*ap       # [N,1]
    out = logsumexp(f, axis=0)         # [1]

Strategy: shard N=262144 rows across 8 cores (32768 rows each).  Each core
streams its 3x16MB f32 shard through SBUF (SWDGE DMA casts to bf16 in the
datapath), computes per-row dots on DVE (bf16 TT 2x mode + fold-then-reduce),
and reduces its 32768 f-values to per-partition (max, sum-exp) partials in
two column groups so the first group's tail overlaps streaming.  Output per
core is [128, 4] = (m1, s1, m2, s2) per partition.  Host combines the
8*128*2 partials into the final logsumexp — no on-chip collective needed.
"""

import numpy as np

import concourse.bacc as bacc
import concourse.bass as bass
import concourse.tile as tile
from concourse import mybir
from concourse.bass_utils import run_bass_kernel_spmd

N, D = 262144, 128
NCORES = 8
N_LOCAL = N // NCORES            # 32768 rows per core
P = 128                          # partitions
ROWS_PER_PART = N_LOCAL // P     # 256 rows owned by each partition
F32 = mybir.dt.float32
BF16 = mybir.dt.bfloat16
CDT = BF16                       # on-chip compute dtype (DMA casts f32->bf16)

# row-chunk schedule (rows per partition per DMA load): small first chunks so
# compute starts early, 16-row (1MB) loads steady-state.
CHUNKS = [8, 8] + [16] * 15
assert sum(CHUNKS) == ROWS_PER_PART
N_GROUPS = 2                     # logsumexp column groups (tail overlap)
GROUP_COLS = ROWS_PER_PART // N_GROUPS


def _build(c1: float, c2: float) -> bass.Bass:
    nc = bacc.Bacc()
    a_ext = nc.declare_dram_parameter("anchor", [N_LOCAL, D], F32, isOutput=False)
    p_ext = nc.declare_dram_parameter("positive", [N_LOCAL, D], F32, isOutput=False)
    n_ext = nc.declare_dram_parameter("negative", [N_LOCAL, D], F32, isOutput=False)
    out_ext = nc.declare_dram_parameter("out", [P, 2 * N_GROUPS], F32, isOutput=True)

    # Partition p owns rows [p*256, (p+1)*256): contiguous 128KB per partition
    # in DRAM -> fully coalesced DMA descriptors.
    a_v = a_ext.rearrange("(p r) d -> p r d", p=P)
    p_v = p_ext.rearrange("(p r) d -> p r d", p=P)
    n_v = n_ext.rearrange("(p r) d -> p r d", p=P)

    with tile.TileContext(nc) as tc:
        with (
            tc.tile_pool(name="ina", bufs=6) as pool_a,
            tc.tile_pool(name="inp", bufs=6) as pool_p,
            tc.tile_pool(name="inn", bufs=6) as pool_n,
            tc.tile_pool(name="q", bufs=4) as pool_q,
            tc.tile_pool(name="t", bufs=4) as pool_t,
            tc.tile_pool(name="u", bufs=4) as pool_u,
            tc.tile_pool(name="fold", bufs=4) as pool_f,
            tc.tile_pool(name="acc", bufs=1) as pool_acc,
        ):
            apn = pool_acc.tile([P, ROWS_PER_PART], F32)
            ap = pool_acc.tile([P, ROWS_PER_PART], F32)
            f = pool_acc.tile([P, ROWS_PER_PART], F32)
            tmp = pool_acc.tile([P, ROWS_PER_PART], F32)
            ms = pool_acc.tile([P, 2 * N_GROUPS], F32)
            negm = pool_acc.tile([P, N_GROUPS], F32)
            expf = pool_acc.tile([P, ROWS_PER_PART], F32)

            def logsumexp_group(g):
                cols = slice(g * GROUP_COLS, (g + 1) * GROUP_COLS)
                # f = c1*apn + c2*ap for this group's columns
                nc.vector.tensor_scalar_mul(tmp[:, cols], ap[:, cols], c2)
                nc.vector.scalar_tensor_tensor(
                    f[:, cols], apn[:, cols], c1, tmp[:, cols],
                    op0=mybir.AluOpType.mult, op1=mybir.AluOpType.add,
                )
                nc.vector.tensor_reduce(
                    out=ms[:, 2 * g : 2 * g + 1], in_=f[:, cols],
                    axis=mybir.AxisListType.X, op=mybir.AluOpType.max,
                )
                nc.vector.tensor_scalar_mul(
                    negm[:, g : g + 1], ms[:, 2 * g : 2 * g + 1], -1.0
                )
                nc.scalar.activation(
                    out=expf[:, cols], in_=f[:, cols],
                    func=mybir.ActivationFunctionType.Exp,
                    bias=negm[:, g : g + 1], scale=1.0,
                    accum_out=ms[:, 2 * g + 1 : 2 * g + 2],
                )

            col = 0
            group_done = 0
            for chunk in CHUNKS:
                B = chunk
                ta = pool_a.tile([P, B * D], CDT, tag="ina")
                tp = pool_p.tile([P, B * D], CDT, tag="inp")
                tn = pool_n.tile([P, B * D], CDT, tag="inn")
                rows = slice(col, col + B)
                # SWDGE (gpsimd) DMA casts f32 -> bf16 in the datapath.
                nc.gpsimd.dma_start(out=ta[:], in_=a_v[:, rows, :])
                nc.gpsimd.dma_start(out=tp[:], in_=p_v[:, rows, :])
                nc.gpsimd.dma_start(out=tn[:], in_=n_v[:, rows, :])

                tq = pool_q.tile([P, B * D], CDT, tag="q")
                nc.vector.tensor_tensor(tq[:], ta[:], tp[:], mybir.AluOpType.add)

                tt = pool_t.tile([P, B * D], CDT, tag="t")
                tu = pool_u.tile([P, B * D], CDT, tag="u")
                nc.vector.tensor_tensor(tt[:], tq[:], tn[:], mybir.AluOpType.mult)
                nc.vector.tensor_tensor(tu[:], ta[:], tp[:], mybir.AluOpType.mult)

                # Per-row dots on DVE: fold halves (bf16 TT 2x) then grouped
                # X-reduce into this chunk's accumulator columns.
                for dst, src in ((apn, tt), (ap, tu)):
                    v3 = src[:].rearrange("p (r d) -> p r d", d=D)
                    tf = pool_f.tile([P, B * (D // 2)], CDT, tag="fold")
                    nc.vector.tensor_tensor(
                        tf[:],
                        v3[:, :, : D // 2],
                        v3[:, :, D // 2 :],
                        mybir.AluOpType.add,
                    )
                    nc.vector.tensor_reduce(
                        out=dst[:, col : col + B],
                        in_=tf[:].rearrange("p (r d) -> p r d", d=D // 2),
                        axis=mybir.AxisListType.X,
                        op=mybir.AluOpType.add,
                    )
                col += B
                # close out any fully-accumulated logsumexp group
                while group_done < N_GROUPS and col >= (group_done + 1) * GROUP_COLS:
                    logsumexp_group(group_done)
                    group_done += 1

            nc.sync.dma_start(out=out_ext[:], in_=ms[:])
    nc.compile()
    return nc


def kernel(anchor, positive, negative, alpha):
    anchor = np.ascontiguousarray(np.asarray(anchor, dtype=np.float32))
    positive = np.ascontiguousarray(np.asarray(positive, dtype=np.float32))
    negative = np.ascontiguousarray(np.asarray(negative, dtype=np.float32))
    a_rad = 2.0 * np.pi * float(np.asarray(alpha)) / 360.0
    t2 = float(np.tan(a_rad) ** 2)
    c1 = 4.0 * t2
    c2 = -2.0 * (1.0 + t2)

    nc = _build(c1, c2)
    in_maps = []
    for i in range(NCORES):
        sl = slice(i * N_LOCAL, (i + 1) * N_LOCAL)
        in_maps.append(
            {"anchor": anchor[sl], "positive": positive[sl], "negative": negative[sl]}
        )
    res = run_bass_kernel_spmd(nc, in_maps, core_ids=list(range(NCORES)))

    ms = np.concatenate([np.asarray(r["out"]) for r in res.results], axis=0)
    m = ms[:, 0::2].reshape(-1).astype(np.float64)
    s = ms[:, 1::2].reshape(-1).astype(np.float64)
    M = m.max()
    S = np.sum(s * np.exp(m - M))
    return np.array([np.log(S) + M], dtype=np.float32)


if __name__ == "__main__":
    rng = np.random.default_rng(0)
    out = kernel(
        anchor=rng.standard_normal((N, D), dtype=np.float32),
        positive=rng.standard_normal((N, D), dtype=np.float32),
        negative=rng.standard_normal((N, D), dtype=np.float32),
        alpha=np.int64(45),
    )
    print("kernel out:", out)


# revision 23
# speedup vs baseline: 1.1359x; 1.1359x over previous
"""AngularLoss Trainium2 kernel (8 NeuronCores, SPMD data-parallel).

Computation (reference):
    t2  = tan(alpha_deg * pi/180)^2
    apn = rowsum((a + p) * n)          # [N,1]
    ap  = rowsum(a * p)                # [N,1]
    f   = 4*t2*apn - 2*(1+t2)*ap       # [N,1]
    out = logsumexp(f, axis=0)         # [1]

Strategy: shard N=262144 rows across 8 cores (32768 rows each).  Each core
streams its 3x16MB f32 shard through SBUF (SWDGE DMA casts to bf16 in the
datapath), computes per-row dots on DVE (bf16 TT 2x mode + fold-then-reduce),
and reduces its 32768 f-values to per-partition (max, sum-exp) partials in
two column groups so the first group's tail overlaps streaming.  Output per
core is [128, 4] = (m1, s1, m2, s2) per partition.  Host combines the
8*128*2 partials into the final logsumexp — no on-chip collective needed.
"""

import numpy as np

import concourse.bacc as bacc
import concourse.bass as bass
import concourse.tile as tile
from concourse import mybir
from concourse.bass_utils import run_bass_kernel_spmd

N, D = 262144, 128
NCORES = 8
N_LOCAL = N // NCORES            # 32768 rows per core
P = 128                          # partitions
ROWS_PER_PART = N_LOCAL // P     # 256 rows owned by each partition
F32 = mybir.dt.float32
BF16 = mybir.dt.bfloat16
CDT = BF16                       # on-chip compute dtype (DMA casts f32->bf16)

# row-chunk schedule (rows per partition per DMA load): small first chunks so
# compute starts early, 16-row (1MB) loads steady-state.
CHUNKS = [8, 8] + [16] * 15
assert sum(CHUNKS) == ROWS_PER_PART
N_GROUPS = 2                     # logsumexp column groups (tail overlap)
GROUP_COLS = ROWS_PER_PART // N_GROUPS


def _build(c1: float, c2: float) -> bass.Bass:
    nc = bacc.Bacc()
    a_ext = nc.declare_dram_parameter("anchor", [N_LOCAL, D], F32, isOutput=False)
    p_ext = nc.declare_dram_parameter("positive", [N_LOCAL, D], F32, isOutput=False)
    n_ext = nc.declare_dram_parameter("negative", [N_LOCAL, D], F32, isOutput=False)
    out_ext = nc.declare_dram_parameter("out", [P, 2 * N_GROUPS], F32, isOutput=True)

    # Partition p owns rows [p*256, (p+1)*256): contiguous 128KB per partition
    # in DRAM -> fully coalesced DMA descriptors.
    a_v = a_ext.rearrange("(p r) d -> p r d", p=P)
    p_v = p_ext.rearrange("(p r) d -> p r d", p=P)
    n_v = n_ext.rearrange("(p r) d -> p r d", p=P)

    with tile.TileContext(nc) as tc:
        with (
            tc.tile_pool(name="ina", bufs=6) as pool_a,
            tc.tile_pool(name="inp", bufs=6) as pool_p,
            tc.tile_pool(name="inn", bufs=6) as pool_n,
            tc.tile_pool(name="q", bufs=4) as pool_q,
            tc.tile_pool(name="t", bufs=4) as pool_t,
            tc.tile_pool(name="u", bufs=4) as pool_u,
            tc.tile_pool(name="fold", bufs=4) as pool_f,
            tc.tile_pool(name="acc", bufs=1) as pool_acc,
        ):
            # Per-group accumulators: separate tiles so a finished group's
            # logsumexp reads never conflict (tile-granularity deps) with the
            # still-streaming group's writes.
            apn_g = [
                pool_acc.tile([P, GROUP_COLS], F32, name=f"apn{g}", tag=f"apn{g}")
                for g in range(N_GROUPS)
            ]
            ap_g = [
                pool_acc.tile([P, GROUP_COLS], F32, name=f"ap{g}", tag=f"ap{g}")
                for g in range(N_GROUPS)
            ]
            f_g = [
                pool_acc.tile([P, GROUP_COLS], F32, name=f"f{g}", tag=f"f{g}")
                for g in range(N_GROUPS)
            ]
            tmp_g = [
                pool_acc.tile([P, GROUP_COLS], F32, name=f"tmp{g}", tag=f"tmp{g}")
                for g in range(N_GROUPS)
            ]
            expf_g = [
                pool_acc.tile([P, GROUP_COLS], F32, name=f"expf{g}", tag=f"expf{g}")
                for g in range(N_GROUPS)
            ]
            negm_g = [
                pool_acc.tile([P, 1], F32, name=f"negm{g}", tag=f"negm{g}") for g in range(N_GROUPS)
            ]
            ms_g = [
                pool_acc.tile([P, 2], F32, name=f"ms{g}", tag=f"ms{g}") for g in range(N_GROUPS)
            ]

            def logsumexp_group(g):
                apn, ap, f, tmp = apn_g[g], ap_g[g], f_g[g], tmp_g[g]
                # f = c1*apn + c2*ap
                nc.vector.tensor_scalar_mul(tmp[:], ap[:], c2)
                nc.vector.scalar_tensor_tensor(
                    f[:], apn[:], c1, tmp[:],
                    op0=mybir.AluOpType.mult, op1=mybir.AluOpType.add,
                )
                nc.vector.tensor_reduce(
                    out=ms_g[g][:, 0:1], in_=f[:],
                    axis=mybir.AxisListType.X, op=mybir.AluOpType.max,
                )
                nc.vector.tensor_scalar_mul(negm_g[g][:], ms_g[g][:, 0:1], -1.0)
                nc.scalar.activation(
                    out=expf_g[g][:], in_=f[:],
                    func=mybir.ActivationFunctionType.Exp,
                    bias=negm_g[g][:], scale=1.0,
                    accum_out=ms_g[g][:, 1:2],
                )

            col = 0
            group_done = 0
            for chunk in CHUNKS:
                B = chunk
                ta = pool_a.tile([P, B * D], CDT, tag="ina")
                tp = pool_p.tile([P, B * D], CDT, tag="inp")
                tn = pool_n.tile([P, B * D], CDT, tag="inn")
                rows = slice(col, col + B)
                # SWDGE (gpsimd) DMA casts f32 -> bf16 in the datapath.
                nc.gpsimd.dma_start(out=ta[:], in_=a_v[:, rows, :])
                nc.gpsimd.dma_start(out=tp[:], in_=p_v[:, rows, :])
                nc.gpsimd.dma_start(out=tn[:], in_=n_v[:, rows, :])

                tq = pool_q.tile([P, B * D], CDT, tag="q")
                nc.vector.tensor_tensor(tq[:], ta[:], tp[:], mybir.AluOpType.add)

                tt = pool_t.tile([P, B * D], CDT, tag="t")
                tu = pool_u.tile([P, B * D], CDT, tag="u")
                nc.vector.tensor_tensor(tt[:], tq[:], tn[:], mybir.AluOpType.mult)
                nc.vector.tensor_tensor(tu[:], ta[:], tp[:], mybir.AluOpType.mult)

                # Per-row dots on DVE: fold halves (bf16 TT 2x) then grouped
                # X-reduce into this chunk's accumulator columns.
                g = col // GROUP_COLS
                gcol = col - g * GROUP_COLS
                assert gcol + B <= GROUP_COLS
                for dst, src in ((apn_g[g], tt), (ap_g[g], tu)):
                    v3 = src[:].rearrange("p (r d) -> p r d", d=D)
                    tf = pool_f.tile([P, B * (D // 2)], CDT, tag="fold")
                    nc.vector.tensor_tensor(
                        tf[:],
                        v3[:, :, : D // 2],
                        v3[:, :, D // 2 :],
                        mybir.AluOpType.add,
                    )
                    nc.vector.tensor_reduce(
                        out=dst[:, gcol : gcol + B],
                        in_=tf[:].rearrange("p (r d) -> p r d", d=D // 2),
                        axis=mybir.AxisListType.X,
                        op=mybir.AluOpType.add,
                    )
                col += B
                # close out any fully-accumulated logsumexp group
                while group_done < N_GROUPS and col >= (group_done + 1) * GROUP_COLS:
                    logsumexp_group(group_done)
                    group_done += 1

            for g in range(N_GROUPS):
                nc.sync.dma_start(
                    out=out_ext[:, 2 * g : 2 * g + 2], in_=ms_g[g][:]
                )
    nc.compile()
    return nc


def kernel(anchor, positive, negative, alpha):
    anchor = np.ascontiguousarray(np.asarray(anchor, dtype=np.float32))
    positive = np.ascontiguousarray(np.asarray(positive, dtype=np.float32))
    negative = np.ascontiguousarray(np.asarray(negative, dtype=np.float32))
    a_rad = 2.0 * np.pi * float(np.asarray(alpha)) / 360.0
    t2 = float(np.tan(a_rad) ** 2)
    c1 = 4.0 * t2
    c2 = -2.0 * (1.0 + t2)

    nc = _build(c1, c2)
    in_maps = []
    for i in range(NCORES):
        sl = slice(i * N_LOCAL, (i + 1) * N_LOCAL)
        in_maps.append(
            {"anchor": anchor[sl], "positive": positive[sl], "negative": negative[sl]}
        )
    res = run_bass_kernel_spmd(nc, in_maps, core_ids=list(range(NCORES)))

    ms = np.concatenate([np.asarray(r["out"]) for r in res.results], axis=0)
    m = ms[:, 0::2].reshape(-1).astype(np.float64)
    s = ms[:, 1::2].reshape(-1).astype(np.float64)
    M = m.max()
    S = np.sum(s * np.exp(m - M))
    return np.array([np.log(S) + M], dtype=np.float32)


if __name__ == "__main__":
    rng = np.random.default_rng(0)
    out = kernel(
        anchor=rng.standard_normal((N, D), dtype=np.float32),
        positive=rng.standard_normal((N, D), dtype=np.float32),
        negative=rng.standard_normal((N, D), dtype=np.float32),
        alpha=np.int64(45),
    )
    print("kernel out:", out)
